# revision 51
# baseline (speedup 1.0000x reference)
"""Trainium2 Bass kernel: multi-head attention (B=2, S=2048, D=1024, H=16,
d_k=64) with RoPE and masked softmax, sharded over 8 NeuronCores as
(batch x head-group): core = b*4 + g handles batch b, heads [4g, 4g+4).

Per-core device program. HW model (measured): the PE clock is DVFS'd —
1.2 GHz "cold" vs 2.4 GHz "warm"; sustained full-array (K>=96) matmul
streams hold it warm, while K=64 matmuls leave it cold and double the
cost of EVERYTHING sharing the window. Warm per-matmul cost is about
(398 + N)/2.4 ns for N moving columns. The schedule therefore keeps a
dense stream of K=128 matmuls from first instruction to last:

  1. Inputs: x and QKV weights ship bf16; partial outputs ship bf16 and
     the host sums the 4 group partials in f32. v ships host-pre-swizzled
     (xvS) so each s-tile column slab is one contiguous-descriptor DMA.
  2. A warm-up burst of dummy matmuls runs during the otherwise-idle
     input-DMA window so the PE clock is already 2.4 GHz when the first
     projection lands.
  3. Only the FIRST s-half of Q-t0/K-t0 uses a DUAL projection: W x and
     (P W) x with rotate-half P folded into a second host-side weight
     copy (sign in the sin table), so RoPE = main*cos + rot*sin straight
     from PSUM. Everything else projects classically; rotate-DMA chains
     and DVE muls run under early attention.
  4. SCORES RUN AT K=128: Q is stored in four per-head [128, S] strips
     (head data in its 64 partitions, ZEROS in the other 64), and the
     score matmul takes the PACKED two-head K tile as lhsT against the
     Q strip as rhs — the other head's K rows meet zeros and contribute
     nothing. This doubles the score column rate AND holds the warm
     clock through the attention phase. K stays packed (no extra space).
  5. The V projection is fused into the attention stream (one s-tile
     just ahead of use, PSUM borrowed from the ctx pool, ACT evac). A
     ones column per 65-wide V head block makes the PV matmul emit
     softmax denominators for free (M=65). exp on ACT over exact causal
     ranges; DVE multiplies the diagonal 128-block of the exp OUTPUT by
     a host 0/1 triangle. ctx^T accumulates f32 over k-tiles; the kt
     loop is software-pipelined (PV lags 2/3) so the PE stream never
     parks behind exp.
  6. 1/sumexp via a 3-hop: DMA spreads the PSUM sum row [1,1024] across
     [64,16], DVE reciprocal (16 elems/lane instead of 1024 on one
     lane, ~60x faster), DMA gathers back to [1,1024], SP-issued
     0-stride DMA broadcasts across 64 partitions, DVE applies it while
     evacuating ctx (bf16). W_o is bf16 (fp32r streams at half rate and
     poisons FWL); qc0's W_o tiles are emitted right after qc0's heads
     so they fill qc1's exp-bound gaps, the rest after attention.

Engine roles: SP = input loads + rb broadcast; Pool = const loads, t1
rotate DMAs, output stores; ACT = exp, Q/K evacs, V evacs, half the Wo
evacs; DVE = everything else. Wait counts >1 are hoisted onto
single-wait no-ops after scheduling (walrus codegen limitation)."""
import sys

sys.path.insert(0, "/opt/trn_rl_repo")

from contextlib import ExitStack

import numpy as np

import concourse.bass as bass
import concourse.mybir as mybir
import concourse.tile as tile
FP = mybir.dt.float32
FPR = mybir.dt.float32r
BF = mybir.dt.bfloat16
EXP = mybir.ActivationFunctionType.Exp

D = 1024        # d_model
S = 2048        # sequence length
NB = 2          # batches
HPG = 4         # heads per group (= per core)
DK = 64         # head dim
F = HPG * DK    # 256 = group feature width
KT = D // 128   # 8 contraction tiles for projections
ST = S // 128   # 16 seq tiles
QCW = 1024      # q-chunk width (= 2 PSUM banks)
NQC = S // QCW  # 2
NEG = -1e9

_nc_cache = {}

# rotate-half column permutation for the dual K projection: within each
# head's 64-wide dk block, swap the 32-halves (sign is folded into sin)
_ROT_PERM = np.array([64 * (c // 64) + ((c % 64) ^ 32) for c in range(F)])


def _mm(nc, out, lhsT, rhs, **kw):
    nc.tensor.matmul(out, lhsT, rhs, **kw)


def _hoist_waits(nc):
    """Several walrus codegen structs (fused-LDW fp32/fp32r matmul, pseudo
    direct2d DMA, ...) only have room for a single sync wait. Hoist every
    limited instruction's waits (when >1) onto same-engine no-ops inserted
    just before it."""
    f = nc.m.functions[0]

    def engine_builder(eng):
        return {
            mybir.EngineType.PE: nc.tensor,
            mybir.EngineType.DVE: nc.vector,
            mybir.EngineType.Activation: nc.scalar,
            mybir.EngineType.Pool: nc.gpsimd,
            mybir.EngineType.SP: nc.sync,
        }[eng]

    def fresh_nop(eng):
        inst = engine_builder(eng).nop().ins
        for b in f.blocks:
            for i, x in enumerate(b.instructions):
                if x is inst:
                    del b.instructions[i]
                    return inst
        raise RuntimeError("created nop not found in any block")

    total = 0
    for blk in f.blocks:
        out = []
        for inst in blk.instructions:
            si = inst.sync_info
            if si is not None and len(si.on_wait) > 1:
                for w in si.on_wait[:-1]:
                    nop = fresh_nop(inst.engine)
                    nop.sync_info = mybir.SyncInfo(on_wait=[w], on_update=[])
                    out.append(nop)
                    total += 1
                inst.sync_info = mybir.SyncInfo(on_wait=[si.on_wait[-1]],
                                                on_update=list(si.on_update))
            out.append(inst)
        blk.instructions[:] = out
    return total


def _causal_ranges(qc, kt):
    """Per (q-chunk, k-tile) causal sub-ranges.

    Returns (j0, a0, a1) where j0 is the first unmasked q column, and
    [a0:512] / [a1:1024] are the bank-0/1 compute ranges (a >= 512 or
    >= 1024 means the bank is skipped). a is pulled below j0 only to keep
    the matmul moving dim >= 256 (fp32r full-rate threshold); the [a:j0)
    gap is zeroed in e_t by Pool."""
    j0 = max(0, kt * 128 - qc * QCW)
    a0 = j0 if j0 < 512 else 512
    a1 = max(j0, 512) if j0 < 1024 else 1024
    return j0, a0, a1


def build_nc(mask_mode):
    """mask_mode: 'causal' | 'full' | 'general'."""
    assert mask_mode in ("causal", "full", "general")
    nc = bass.Bass("TRN2", target_bir_lowering=False, debug=False, num_devices=8)

    # x and the QKV projection weights ship as bf16: the projection matmuls
    # run at the same PE rate as fp32r but input DMA bytes halve (phase 1 is
    # DMA-bandwidth-bound at fp32); everything downstream stays fp32.
    xqT = nc.dram_tensor("xqT", [D, S], BF, kind="ExternalInput").ap()
    xkT = nc.dram_tensor("xkT", [D, S], BF, kind="ExternalInput").ap()
    # v input pre-swizzled host-side so each s-tile's column slab
    # [128, KT*128] is one contiguous DMA (2KB/partition descriptors)
    xvS = nc.dram_tensor("xvS", [ST * 128, KT * 128], BF, kind="ExternalInput").ap()
    # weights ship host-pre-swizzled to the exact SBUF layout
    # ([p, k*F+f] / [p, t*D+e]) so each load is one contiguous
    # 4KB-per-partition descriptor instead of 1024 strided 512B ones
    wqT = nc.dram_tensor("wqT", [128, KT * F], BF, kind="ExternalInput").ap()
    wkT = nc.dram_tensor("wkT", [128, KT * F], BF, kind="ExternalInput").ap()
    wqrT = nc.dram_tensor("wqrT", [128, KT * F], BF, kind="ExternalInput").ap()
    wkrT = nc.dram_tensor("wkrT", [128, KT * F], BF, kind="ExternalInput").ap()
    wvT = nc.dram_tensor("wvT", [128, KT * F], BF, kind="ExternalInput").ap()
    woT = nc.dram_tensor("woT", [128, 2 * D], BF, kind="ExternalInput").ap()
    # bf16 tables: rope muls on SBUF operands then hit DVE's 2x bf16
    # mode (fp32 tables force 1x); table rounding ~0.4% matches the bf16
    # activation rounding already present
    cosd = nc.dram_tensor("cosS", [128, S], BF, kind="ExternalInput").ap()
    sind = nc.dram_tensor("sinS", [128, S], BF, kind="ExternalInput").ap()
    if mask_mode == "general":
        biasT = nc.dram_tensor("biasT", [S, S], FP, kind="ExternalInput").ap()
    if mask_mode == "causal":
        # [128, 256] 0/1 mask: zero left half, lower-triangle right half —
        # one Pool multiply masks both the diagonal block and the <=128-col
        # gap kept only for matmul width
        triD = nc.dram_tensor("triD", [128, 256], BF, kind="ExternalInput").ap()
    # partial output ships bf16 (host sums the 4 group partials in f32)
    outp = nc.dram_tensor("outp", [S, D], BF, kind="ExternalOutput").ap()

    with tile.TileContext(nc) as tc, ExitStack() as ctx:
        const = ctx.enter_context(tc.tile_pool(name="const", bufs=1))
        qk = ctx.enter_context(tc.tile_pool(name="qk", bufs=1))

        wq_sb = const.tile([128, KT * F], BF)
        wk_sb = const.tile([128, KT * F], BF)
        wqr_sb = const.tile([128, KT * F], BF)
        wkr_sb = const.tile([128, KT * F], BF)
        wv_sb = const.tile([128, KT * F], BF)
        wo_sb = const.tile([128, 2 * D], BF)
        cos_sb = const.tile([128, S], BF)
        sin_sb = const.tile([128, S], BF)
        ones64 = const.tile([128, 64], FP)
        # bf16 ones row for the K=1 broadcast matmul in the normalize tail
        ones_bf = const.tile([1, 64], BF)
        # single-DMA weight loads (multiple DMAs into one tile would attach
        # too many sem waits to the first fused-LDW matmul for walrus);
        # issued from Pool/SWDGE so ACT/SP queues stay clear
        nc.gpsimd.dma_start(wq_sb[:], wqT[:])
        nc.gpsimd.dma_start(wk_sb[:], wkT[:])
        nc.gpsimd.dma_start(wqr_sb[:], wqrT[:])
        nc.gpsimd.dma_start(wkr_sb[:], wkrT[:])
        if mask_mode == "causal":
            tri_sb = const.tile([128, 256], BF)

        # persistent activations. K: packed [p, t*S + s] (t-tile 0: heads
        # 0,1; t-tile 1: heads 2,3). Q: four per-head STRIPS [128, S] at
        # cols [h*S:(h+1)*S] — head data in its 64 partitions (po), ZEROS
        # in the other 64, so score matmuls run K=128 against the packed
        # K pair (other head's K rows meet zeros).
        qt_sb = qk.tile([128, HPG * S], BF)
        kt_sb = qk.tile([128, 2 * S], BF)
        # V in [s, f] layout with a ones column per head: 65-wide head blocks
        v_sb = qk.tile([128, ST * HPG * 65], BF)
        ctxn_sb = qk.tile([128, 2 * S], BF)

        # warm-up burst: the PE DVFS clock starts at 1.2 GHz and only
        # reaches 2.4 GHz under a sustained full-array matmul stream.
        # Fill the otherwise-idle input-DMA window (~17us) with dummy
        # matmuls so the first projection already runs warm. The wu
        # memset is the FIRST DVE op so nothing delays the burst.
        wu_sb = qk.tile([128, 512], BF, name="wu")
        nc.vector.memset(wu_sb[:], 0.5)
        nc.vector.memset(ones64[:], 1.0)
        nc.vector.memset(ones_bf[:], 1.0)

        # zero the dead half of each Q strip once (Pool: off the DVE
        # critical path; needed only by the first scores at ~40us)
        for h_ in range(HPG):
            po_ = (h_ % 2) * 64
            nc.gpsimd.memset(qt_sb[64 - po_:128 - po_, h_ * S:(h_ + 1) * S], 0.0)

        # xv pool outlives phase 1: V projection is fused into the
        # attention stream (one s-tile just ahead of the scores that use it)
        xvpool = ctx.enter_context(tc.tile_pool(name="xvs", bufs=4))

        # x / rope / scratch pools outlive phase 1: the t1 projections are
        # emitted INSIDE the attention stream (between heads (0,1) and
        # (1,0)) so the first scores do not queue behind them on the
        # in-order PE
        xpool = ctx.enter_context(tc.tile_pool(name="xs", bufs=16))
        rpool = ctx.enter_context(tc.tile_pool(name="rope", bufs=3))
        qscr = ctx.enter_context(tc.tile_pool(name="qscr", bufs=1))

        # ---------------- phase 1: projections + RoPE ----------------
        with ExitStack() as pctx:
            pps = pctx.enter_context(tc.tile_pool(name="pps", bufs=4, space="PSUM"))

            # warm-up: dummy 512-col matmuls from ~8.7us (after the PE
            # preamble) until the first x tile lands (~15us). The result is
            # read once into wu_sb so walrus cannot dead-code-eliminate the
            # burst.
            wu_ps = pps.tile([128, 1024], FP, tag="pj", name="wu_ps")
            for _wu in range(56):
                _mm(nc, wu_ps[:, 0:512], wu_sb[:, 0:128], wu_sb[:],
                    start=True, stop=True)
            nc.vector.tensor_copy(wu_sb[0:1, 0:8], wu_ps[0:1, 0:8])

            # issue all xq then all xk loads upfront: K's tiles are resident
            # the moment Q's PSUM accumulators free up — attention gates on
            # K-t0's RoPE, so everything on that path is prioritized
            x_tiles = {}
            for nm, x_d in (("q", xqT), ("k", xkT)):
                for k in range(KT):
                    xt = xpool.tile([128, S], BF, tag="x", name=f"x{nm}{k}")
                    nc.sync.dma_start(xt[:], x_d[k * 128:(k + 1) * 128, :])
                    x_tiles[(nm, k)] = xt
            # late consts: cos/sin aren't needed before the first dual evac
            # (~28us) and wv/tri/wo before ~35us — issuing them after the x
            # loads keeps their transfers out of the DMA queue ahead of the
            # critical first x tiles (x k0 gates the first projection)
            nc.gpsimd.dma_start(cos_sb[:], cosd[:])
            nc.gpsimd.dma_start(sin_sb[:], sind[:])
            nc.sync.dma_start(wv_sb[:], wvT[:])
            if mask_mode == "causal":
                nc.sync.dma_start(tri_sb[:], triD[:])
            nc.sync.dma_start(wo_sb[:], woT[:])

            def rope(dst_sb, t, eng):
                """eng: DVE (critical path) or Pool (off-path t1 tiles)."""
                lo, hi = t * S, (t + 1) * S
                rot = rpool.tile([128, S], BF, tag="rot")
                # rotate-half across partitions: [0:32]<-[32:64],
                # [32:64]<-[0:32], [64:96]<-[96:128], [96:128]<-[64:96]
                for dst0, src0 in ((0, 32), (32, 0), (64, 96), (96, 64)):
                    nc.gpsimd.dma_start(rot[dst0:dst0 + 32, :],
                                        dst_sb[src0:src0 + 32, lo:hi])
                eng.tensor_mul(rot[:], rot[:], sin_sb[:])
                eng.tensor_mul(dst_sb[:, lo:hi], dst_sb[:, lo:hi], cos_sb[:])
                eng.tensor_add(dst_sb[:, lo:hi], dst_sb[:, lo:hi], rot[:])

            def rope_range(dst_sb, lo, w, eng, co=None):
                """rotate-half rope over dst_sb[:, lo:lo+w] via 4 rot DMAs
                (a WAW chain — keep off the critical path) + mul/mul/add."""
                if co is None:
                    co = lo % S
                rot = rpool.tile([128, S], BF, tag="rot")
                for dst0, src0 in ((0, 32), (32, 0), (64, 96), (96, 64)):
                    nc.gpsimd.dma_start(rot[dst0:dst0 + 32, 0:w],
                                        dst_sb[src0:src0 + 32, lo:lo + w])
                eng.tensor_mul(rot[:, 0:w], rot[:, 0:w], sin_sb[:, co:co + w])
                eng.tensor_mul(dst_sb[:, lo:lo + w], dst_sb[:, lo:lo + w],
                               cos_sb[:, co:co + w])
                eng.tensor_add(dst_sb[:, lo:lo + w], dst_sb[:, lo:lo + w],
                               rot[:, 0:w])

            def dual_half(nm, w_sb, wr_sb, dst_sb, t, half):
                """DUAL projection of ONE s-half of a t-pair: W x AND (P W) x
                with the rotate-half permutation P folded into a second
                host-side weight copy, so RoPE = main*cos + rot*sin straight
                from PSUM — no rotate-DMA chain. Costs an extra half
                projection of PE; used only for the s-half that gates the
                first scores."""
                m = pps.tile([128, 1024], FP, tag="pj", name=f"m{nm}{t}{half}")
                r = pps.tile([128, 1024], FP, tag="pj", name=f"r{nm}{t}{half}")
                xo = half * 1024
                for k in range(KT):
                    xt = x_tiles[(nm, k)]
                    for sc in range(2):
                        _mm(nc, m[:, sc * 512:(sc + 1) * 512],
                            w_sb[:, k * F + t * 128: k * F + (t + 1) * 128],
                            xt[:, xo + sc * 512: xo + (sc + 1) * 512],
                            start=(k == 0), stop=(k == KT - 1))
                        _mm(nc, r[:, sc * 512:(sc + 1) * 512],
                            wr_sb[:, k * F + t * 128: k * F + (t + 1) * 128],
                            xt[:, xo + sc * 512: xo + (sc + 1) * 512],
                            start=(k == 0), stop=(k == KT - 1))
                # evac in 512-col chunks, returned as closures so the
                # caller can interleave q/k chunks on the DVE queue (the
                # first score matmul needs only q-chunk0 + k-chunk0)
                eng = nc.vector
                scr = rpool.tile([128, S], BF, tag="rot")
                scr2 = None
                if nm == "q":
                    scr2 = rpool.tile([128, S], BF, tag="rot", name="scr2")

                def evac_chunk(c0):
                    eng.tensor_mul(scr[:, c0:c0 + 512], r[:, c0:c0 + 512],
                                   sin_sb[:, xo + c0:xo + c0 + 512])
                    if nm == "q":
                        # per-head strips: head 2t -> strip 2t (data in
                        # partitions 0:64), head 2t+1 -> strip 2t+1
                        eng.tensor_mul(scr2[:, c0:c0 + 512],
                                       m[:, c0:c0 + 512],
                                       cos_sb[:, xo + c0:xo + c0 + 512])
                        for po in (0, 64):
                            so = (2 * t + po // 64) * S + xo + c0
                            eng.tensor_add(dst_sb[po:po + 64, so:so + 512],
                                           scr2[po:po + 64, c0:c0 + 512],
                                           scr[po:po + 64, c0:c0 + 512])
                    else:
                        lo = t * S + xo + c0
                        eng.tensor_mul(dst_sb[:, lo:lo + 512],
                                       m[:, c0:c0 + 512],
                                       cos_sb[:, xo + c0:xo + c0 + 512])
                        eng.tensor_add(dst_sb[:, lo:lo + 512],
                                       dst_sb[:, lo:lo + 512],
                                       scr[:, c0:c0 + 512])
                return evac_chunk

            def strip_copy(dst_sb, qs, t, xo, w, qoff=0):
                """Copy roped packed Q scratch into the per-head strips
                (ACT: it has slack before ~100us; Pool's 1x-rate tensor ops
                would serialize behind its DMA-issue queue)."""
                for po in (0, 64):
                    so = (2 * t + po // 64) * S + xo
                    nc.scalar.copy(dst_sb[po:po + 64, so:so + w],
                                   qs[po:po + 64, qoff:qoff + w])

            def classic_half(nm, w_sb, dst_sb, t, half):
                """Classic projection of one s-half (1 tile = 2 banks) +
                rotate-DMA rope; the chain runs under early attention."""
                ps = pps.tile([128, 1024], FP, tag="pj", name=f"c{nm}{t}{half}")
                xo = half * 1024
                for k in range(KT):
                    xt = x_tiles[(nm, k)]
                    for sc in range(2):
                        _mm(nc, ps[:, sc * 512:(sc + 1) * 512],
                            w_sb[:, k * F + t * 128: k * F + (t + 1) * 128],
                            xt[:, xo + sc * 512: xo + (sc + 1) * 512],
                            start=(k == 0), stop=(k == KT - 1))
                if nm == "q":
                    qs = qscr.tile([128, S], BF, tag="qs")
                    nc.scalar.copy(qs[:, 0:1024], ps[:])
                    rope_range(qs, 0, 1024, nc.vector, co=xo)
                    strip_copy(dst_sb, qs, t, xo, 1024)
                else:
                    lo = t * S + xo
                    nc.scalar.copy(dst_sb[:, lo:lo + 1024], ps[:])
                    rope_range(dst_sb, lo, 1024, nc.vector)

            def classic_t(nm, w_sb, dst_sb, t, pool, ptag):
                """Classic projection of one t-pair (2 tiles = 4 banks) +
                rotate-DMA RoPE; PSUM comes from the caller's pool so this
                can run inside the attention stream."""
                ps = [pool.tile([128, 1024], FP, tag=ptag, name=f"c{nm}{t}{i}")
                      for i in range(2)]
                for k in range(KT):
                    xt = x_tiles[(nm, k)]
                    for sc in range(4):
                        _mm(nc, ps[sc // 2][:, (sc % 2) * 512:(sc % 2) * 512 + 512],
                            w_sb[:, k * F + t * 128: k * F + (t + 1) * 128],
                            xt[:, sc * 512:(sc + 1) * 512],
                            start=(k == 0), stop=(k == KT - 1))
                # per-half chains: the qc0-facing half [0:1024] completes
                # its rope+strip ~3us before the full-width version would,
                # ungating head (0,2) earlier
                if nm == "q":
                    qs = qscr.tile([128, S], BF, tag="qs")
                    for half in range(2):
                        nc.scalar.copy(qs[:, half * 1024:(half + 1) * 1024],
                                       ps[half][:])
                        rope_range(qs, half * 1024, 1024, nc.vector)
                        strip_copy(dst_sb, qs, t, half * 1024, 1024,
                                   qoff=half * 1024)
                else:
                    for half in range(2):
                        lo = t * S + half * 1024
                        nc.scalar.copy(dst_sb[:, lo:lo + 1024], ps[half][:])
                        rope_range(dst_sb, lo, 1024, nc.vector)

            # only the FIRST s-half of each t0 tensor gates the first
            # scores (heads (0,0)/(0,1) read cols [0:1024]) — dual-project
            # that half; the second halves go classic here. The t1 pairs
            # are emitted inside the attention stream (they would
            # otherwise delay the first scores by ~25us of PE queue).
            evq = dual_half("q", wq_sb, wqr_sb, qt_sb, 0, 0)
            evk = dual_half("k", wk_sb, wkr_sb, kt_sb, 0, 0)
            evq(0)
            evk(0)
            evq(512)
            evk(512)
            classic_half("q", wq_sb, qt_sb, 0, 1)
            classic_half("k", wk_sb, kt_sb, 0, 1)
            classic_t_ref = classic_t

        # V: out[s_tile, f] layout via column-slab x loads, one s-tile at a
        # time, emitted from inside the attention loop just ahead of use so
        # the PE stream reaches the first scores ~6us earlier
        _v_done = set()
        ctx_pool_ref = [None]

        def emit_v(st):
            if st in _v_done or st >= ST:
                return
            _v_done.add(st)
            xslab = xvpool.tile([128, KT * 128], BF, tag="xv", name=f"xv{st}")
            nc.sync.dma_start(
                xslab[:], xvS[st * 128:(st + 1) * 128, :])
            pv = ctx_pool_ref[0].tile([128, 1024], FP, tag="ctx", name=f"pv{st}")
            for k in range(KT):
                _mm(nc, pv[:, 0:256], xslab[:, k * 128:(k + 1) * 128],
                    wv_sb[:, k * F:(k + 1) * F],
                    start=(k == 0), stop=(k == KT - 1))
            c0 = st * HPG * 65
            dstv = v_sb[:, c0:c0 + HPG * 65].rearrange(
                "p (h c) -> p h c", h=HPG)[:, :, 0:64]
            srcv = pv[:, 0:256].rearrange("p (h c) -> p h c", h=HPG)
            nc.scalar.copy(dstv, srcv)

        # ---------------- phase 2: attention ----------------
        with ExitStack() as actx:
            sc_ps = actx.enter_context(tc.tile_pool(name="scps", bufs=2, space="PSUM"))
            ctx_ps = actx.enter_context(tc.tile_pool(name="ctxps", bufs=2, space="PSUM"))
            ctx_pool_ref[0] = ctx_ps
            epool = actx.enter_context(tc.tile_pool(name="exp", bufs=6))
            npool = actx.enter_context(tc.tile_pool(name="norm", bufs=2))
            opool = actx.enter_context(tc.tile_pool(name="ost", bufs=4))
            if mask_mode == "general":
                bpool = actx.enter_context(tc.tile_pool(name="bias", bufs=2))

            ones_ap = v_sb[:].rearrange("p (b c) -> p b c", c=65)[:, :, 64:65]
            nc.vector.tensor_copy(ones_ap, ones64[:].rearrange("p (b o) -> p b o", o=1))

            pend_norm = []

            def flush_norm():
                while pend_norm:
                    pend_norm.pop(0)()

            def emit_head(qc, h, last=False):
                if True:
                    t, po = h // 2, (h % 2) * 64
                    kt_hi = 8 * qc + 8 if mask_mode == "causal" else ST
                    last_b0 = min(kt_hi - 1, 8 * qc + 3) if mask_mode == "causal" else ST - 1
                    ctx_t = ctx_ps.tile([128, QCW], FP, tag="ctx")
                    # Q strip for head h: data in partitions [po:po+64],
                    # zeros elsewhere — lets scores run K=128 vs packed K
                    qbase = h * S + qc * QCW

                    def emit_pv(e_t, kt, a0, a1):
                        vcol = kt * HPG * 65 + h * 65
                        if a0 < 512:
                            _mm(nc, ctx_t[0:65, a0:512], v_sb[:, vcol:vcol + 65],
                                e_t[:, a0:512],
                                start=(kt == 0), stop=(kt == last_b0))
                        _mm(nc, ctx_t[0:65, a1:QCW], v_sb[:, vcol:vcol + 65],
                            e_t[:, a1:QCW],
                            start=(kt == 0), stop=(kt == kt_hi - 1))

                    # software pipeline: PV(kt-2) is emitted AFTER scores(kt)
                    # so the in-order PE stream never stalls waiting for
                    # exp(kt-2) with scores work available
                    if h == 0:
                        emit_v(8 * qc)
                        emit_v(8 * qc + 1)
                    pend = []
                    for kt in range(kt_hi):
                        if kt == 4:
                            # previous head's normalize tail: by now its
                            # gather DMA has completed, so the broadcast
                            # matmul slots into the PE stream stall-free
                            flush_norm()
                        if h == 0:
                            emit_v(kt + 2)
                        if mask_mode == "causal":
                            j0, a0, a1 = _causal_ranges(qc, kt)
                        else:
                            j0, a0, a1 = 0, 0, 512
                        kcol = t * S + kt * 128
                        s_ps = sc_ps.tile([128, QCW], FP, tag="sc")
                        if a0 < 512:
                            _mm(nc, s_ps[:, a0:512],
                                kt_sb[:, kcol:kcol + 128],
                                qt_sb[:, qbase + a0:qbase + 512],
                                start=True, stop=True)
                        _mm(nc, s_ps[:, a1:QCW],
                            kt_sb[:, kcol:kcol + 128],
                            qt_sb[:, qbase + a1:qbase + QCW],
                            start=True, stop=True)
                        if mask_mode == "general":
                            bt = bpool.tile([128, QCW], FP, tag="bt")
                            nc.sync.dma_start(
                                bt[:], biasT[kt * 128:(kt + 1) * 128,
                                             qc * QCW:(qc + 1) * QCW])
                            nc.vector.tensor_add(s_ps[:], s_ps[:], bt[:])
                        e_t = epool.tile([128, QCW], BF, tag="e")
                        # exp from the first COMPUTED column (a-range) — the
                        # [a:j0) strip holds real but masked scores, zeroed by
                        # the tri multiply below
                        ax = a0 if a0 < 512 else a1
                        nc.scalar.activation(e_t[:, ax:QCW], s_ps[:, ax:QCW], EXP)
                        if mask_mode == "causal" and kt * 128 >= qc * QCW:
                            if j0 > ax:
                                nc.vector.tensor_mul(e_t[:, ax:ax + 256],
                                                     e_t[:, ax:ax + 256],
                                                     tri_sb[:])
                            else:
                                nc.vector.tensor_mul(e_t[:, j0:j0 + 128],
                                                     e_t[:, j0:j0 + 128],
                                                     tri_sb[:, 128:256])
                        pend.append((e_t, kt, a0, a1))
                        if len(pend) > (2 if qc == 0 else 3):
                            emit_pv(*pend.pop(0))
                    for p_ in pend:
                        emit_pv(*p_)
                    # normalize: rows 0:64 are ctx^T, row 64 is sum(exp).
                    # Evacuate ctx+sum to SBUF in ONE copy so nothing below
                    # holds the accumulating region; then spread the sum row
                    # across 64 partitions (DMA), reciprocal at 16
                    # elems/lane (vs 1024 serial on one lane), gather back
                    # as bf16.
                    ctxe = npool.tile([65, QCW], FP, tag="ce")
                    if last:
                        # final head: the sum row goes via ACT (idle after
                        # its last exp) concurrently with the ctx body on
                        # DVE, shortening the only non-overlapped tail
                        nc.scalar.copy(ctxe[64:65, :], ctx_t[64:65, :])
                        nc.vector.tensor_copy(ctxe[0:64, :], ctx_t[0:64, :])
                    else:
                        nc.vector.tensor_copy(ctxe[:], ctx_t[0:65, :])
                    dq = nc.scalar if last else nc.sync
                    rs_sb = npool.tile([64, 16], FP, tag="rs")
                    dq.dma_start(
                        rs_sb[:],
                        ctxe[64:65, :].rearrange("o (p c) -> o p c", p=64))
                    rr_sb = npool.tile([64, 16], BF, tag="rr")
                    with nc.allow_low_precision(reason="reciprocal"):
                        nc.vector.reciprocal(rr_sb[:], rs_sb[:])
                    r_sb = npool.tile([1, QCW], BF, tag="r")
                    dq.dma_start(
                        r_sb[:].rearrange("o (p c) -> o p c", p=64),
                        rr_sb[:])

                    def norm_tail(ctx_t=ctx_t, ctxe=ctxe, r_sb=r_sb,
                                  po=po, t=t, qc=qc):
                        # broadcast 1/sum across 64 partitions with a K=1
                        # matmul into the (now spare) rows 64:128 of the
                        # head's ctx PSUM tile — a 64-descriptor SBUF->SBUF
                        # broadcast DMA costs ~8us of DMA-engine time; the
                        # PE does it in ~0.8us. Runs one head deferred so
                        # the in-order PE queue never waits on the gather.
                        for bk in range(2):
                            _mm(nc, ctx_t[64:128, bk * 512:(bk + 1) * 512],
                                ones_bf[:], r_sb[:, bk * 512:(bk + 1) * 512],
                                start=True, stop=True)
                        nc.vector.tensor_mul(
                            ctxn_sb[po:po + 64,
                                    t * S + qc * QCW: t * S + (qc + 1) * QCW],
                            ctxe[0:64, :], ctx_t[64:128, :])
                    pend_norm.append(norm_tail)

            def emit_wo(st_lo, st_hi):
                for st in range(st_lo, st_hi):
                    o_ps = sc_ps.tile([128, QCW], FP, tag="sc", name="ops")
                    # ft outer: the two banks of one ft share the same
                    # stationary ctxn block, so ldw-opt skips every second
                    # weight load
                    for ft in range(2):
                        for ec in range(2):
                            _mm(nc, o_ps[:, ec * 512:(ec + 1) * 512],
                                ctxn_sb[:, ft * S + st * 128: ft * S + (st + 1) * 128],
                                wo_sb[:, ft * D + ec * 512: ft * D + (ec + 1) * 512],
                                start=(ft == 0), stop=(ft == 1))
                    o_sb = opool.tile([128, QCW], BF, tag="o")
                    nc.vector.tensor_copy(o_sb[:], o_ps[:])
                    nc.gpsimd.dma_start(
                        outp[st * 128:(st + 1) * 128, :], o_sb[:])

            # t0 heads (0,1) of both q-chunks first: the t1 RoPE finishes
            # while they run, so ACT never idles waiting for heads 2,3.
            # Each head's normalize tail is emitted at the START of the
            # next head (so its gather DMA completes before the PE queue
            # reaches the broadcast matmul); qc0's Wo tiles are emitted one
            # head after qc0 completes for the same reason.
            emit_head(0, 0)
            emit_head(0, 1)
            # t1 projections here: heads (0,0)/(0,1) only read the dual-
            # projected t0 half, so they run first; the t1 rope chains
            # then overlap heads (1,0)/(1,1), ready for (0,2)
            classic_t_ref("q", wq_sb, qt_sb, 1, sc_ps, "sc")
            classic_t_ref("k", wk_sb, kt_sb, 1, sc_ps, "sc")
            for qc, h in ((1, 0), (1, 1), (0, 2), (0, 3), (1, 2)):
                emit_head(qc, h)
            emit_wo(0, ST // 2)
            emit_head(1, 3, last=True)
            flush_norm()
            emit_wo(ST // 2, ST)
    _hoist_waits(nc)
    return nc


def _get_nc(mask_mode):
    if mask_mode not in _nc_cache:
        _nc_cache[mask_mode] = build_nc(mask_mode)
    return _nc_cache[mask_mode]


def _rope_tables():
    """cos/sin tables in [128, S] layout (64-row block tiled twice); sin is
    sign-folded for the rotate-half term."""
    inv_freq = (1.0 / (10000.0 ** (np.arange(0, DK, 2, dtype=np.float32) / np.float32(DK)))).astype(np.float32)
    t = np.arange(S, dtype=np.float32)
    freqs = np.outer(t, inv_freq).astype(np.float32)      # (S, 32)
    emb = np.concatenate([freqs, freqs], axis=-1)         # (S, 64)
    cos64 = np.cos(emb).T.astype(np.float32)              # (64, S)
    sin64 = np.sin(emb).T.astype(np.float32)
    sin64s = sin64.copy()
    sin64s[0:32] = -sin64[0:32]
    cos128 = np.ascontiguousarray(np.tile(cos64, (2, 1)))
    sin128 = np.ascontiguousarray(np.tile(sin64s, (2, 1)))
    return cos128, sin128


def _mask_mode(m2d):
    if (m2d != 0).all():
        return "full"
    if np.array_equal(m2d != 0, np.tril(np.ones((S, S), dtype=bool))):
        return "causal"
    return "general"


def _prepare(inputs):
    q = np.asarray(inputs["query"], dtype=np.float32)
    k = np.asarray(inputs["key"], dtype=np.float32)
    v = np.asarray(inputs["value"], dtype=np.float32)
    mask = np.asarray(inputs["mask"])
    Wq = np.asarray(inputs["W_q"], dtype=np.float32)
    Wk = np.asarray(inputs["W_k"], dtype=np.float32)
    Wv = np.asarray(inputs["W_v"], dtype=np.float32)
    Wo = np.asarray(inputs["W_o"], dtype=np.float32)

    modes = [_mask_mode(mask[b, 0]) for b in range(NB)]
    if all(m == "causal" for m in modes):
        mode = "causal"
    elif all(m == "full" for m in modes):
        mode = "full"
    else:
        mode = "general"
    nc = _get_nc(mode)

    import ml_dtypes
    bf16 = ml_dtypes.bfloat16

    def _wswz(wT):
        # [D, F] -> [128, KT*F]: out[p, k*F+f] = wT[k*128+p, f]
        return np.ascontiguousarray(
            wT.reshape(KT, 128, F).transpose(1, 0, 2).reshape(128, KT * F))

    def _woswz(woT_):
        # [F, D] -> [128, 2*D]: out[p, t*D+e] = woT_[t*128+p, e]
        return np.ascontiguousarray(
            woT_.reshape(2, 128, D).transpose(1, 0, 2).reshape(128, 2 * D))

    cos128, sin128 = _rope_tables()
    scale = np.float32(1.0 / np.sqrt(DK))
    if mode == "causal":
        kk = np.arange(128)[:, None]
        qq = np.arange(128)[None, :]
        tri = np.where(kk <= qq, np.float32(1.0), np.float32(0.0))
        triD = np.concatenate([np.zeros((128, 128), np.float32), tri],
                              axis=1).astype(bf16)

    xT = {}
    biasTs = {}
    for b in range(NB):
        # xvS[st*128+p, k*128+c] = v.T[k*128+p, st*128+c]: each s-tile's
        # column slab is contiguous so the device loads it in one DMA
        vT = v[b].T
        xvS = np.ascontiguousarray(
            vT.reshape(KT, 128, ST, 128).transpose(2, 1, 0, 3)
              .reshape(ST * 128, KT * 128))
        xT[b] = (np.ascontiguousarray(q[b].T).astype(bf16),
                 np.ascontiguousarray(k[b].T).astype(bf16),
                 xvS.astype(bf16))
        if mode == "general":
            biasTs[b] = np.where(mask[b, 0].T != 0, np.float32(0.0),
                                 np.float32(NEG)).astype(np.float32)

    in_maps = []
    for core in range(8):
        b, g = divmod(core, 4)
        rows = slice(g * F, (g + 1) * F)
        m = {
            "xqT": xT[b][0], "xkT": xT[b][1], "xvS": xT[b][2],
            "wqT": _wswz((Wq[rows] * scale).T).astype(bf16),
            "wkT": _wswz(Wk[rows].T).astype(bf16),
            "wqrT": _wswz((Wq[rows] * scale).T[:, _ROT_PERM]).astype(bf16),
            "wkrT": _wswz(Wk[rows].T[:, _ROT_PERM]).astype(bf16),
            "wvT": _wswz(Wv[rows].T).astype(bf16),
            "woT": _woswz(Wo[:, rows].T).astype(bf16),
            "cosS": cos128.astype(bf16), "sinS": sin128.astype(bf16),
        }
        if mode == "general":
            m["biasT"] = biasTs[b]
        if mode == "causal":
            m["triD"] = triD
        in_maps.append(m)
    return nc, in_maps


def _gather(res):
    out = np.zeros((NB, S, D), dtype=np.float32)
    for core in range(8):
        out[core // 4] += np.asarray(res.results[core]["outp"],
                                     dtype=np.float32)
    return out


def kernel(**inputs):
    from concourse import bass_utils

    nc, in_maps = _prepare(inputs)
    res = bass_utils.run_bass_kernel_spmd(nc, in_maps, core_ids=list(range(8)))
    return _gather(res)


def run_traced(**inputs):
    """Run once with NTFF tracing; returns (out, exec_time_ns, raw results)."""
    from concourse import bass_utils

    nc, in_maps = _prepare(inputs)
    res = bass_utils.run_bass_kernel_spmd(nc, in_maps, core_ids=list(range(8)),
                                          trace=True)
    return _gather(res), res.exec_time_ns, res

